# revision 14
# baseline (speedup 1.0000x reference)
"""GAT 2-layer kernel for Trainium2 (Bass/Tile), 8-core SPMD.

Self-contained: host-side packing + bass program build + SPMD run + host
reassembly.  kernel(**inputs) takes the full unsharded inputs and returns
(log_softmax_logits [N,16] f32, alpha2 [E] f32) like the reference.

v2: bulk row gathers via dma_gather (int16 idx -> table split in two
halves), 512B table rows: [h bf16 x128 | 1.0 bf16 | pad | s_src f32 |
s_dst f32 | pad], aggregation matmul in bf16 with f32 PSUM accumulate,
exact per-edge scores in f32, alpha denominators recomputed on host.
"""

import math
import os
import sys

import numpy as np

sys.path.insert(0, "/opt/trn_rl_repo")

import concourse.bacc as bacc
import concourse.bass as bass
import concourse.mybir as mybir
import concourse.tile as tile
from concourse import bass_utils, library_config

P = 128          # partitions / edge-tile size
WIN = 32         # node window (matmul M)
TWU = 256        # table row width in uint16 elems (512 bytes)
COL_ONE_BF = 128     # bf16 elem: constant 1.0 (denominator column)
COL_SSRC_U = 130     # uint16 elems 130:132 = s_src f32
COL_SDST_U = 132     # uint16 elems 132:134 = s_dst f32
RHSW = 129           # matmul rhs bf16 elems: h(0:128) + one(128)
LEAKY = 0.05
GRP = 3              # blocks per gather group
F32 = mybir.dt.float32
BF16 = mybir.dt.bfloat16
U16 = mybir.dt.uint16
I16 = mybir.dt.int16
I32 = mybir.dt.int32


class Packed:
    pass


# ----------------------------------------------------------------------------
# host-side packing
# ----------------------------------------------------------------------------

def pack_edges(src, dst, n_nodes, n_cores, nfeat):
    """Sort edges by src, shard by src-owner core, split per 32-node window
    by dst half (int16 index range), tile into 128-edge tiles.  The tile
    schedule is the max over cores so the SPMD program is identical."""
    pk = Packed()
    N = n_nodes
    E = src.shape[0]
    half = N // 2
    npc = N // n_cores
    assert npc * n_cores == N
    nblk = math.ceil(npc / P)
    wpb = P // WIN
    nwin = nblk * wpb

    perm = np.argsort(src, kind="stable")
    ss = np.asarray(src)[perm].astype(np.int64)
    dsrt = np.asarray(dst)[perm].astype(np.int64)

    win_lo = np.empty(nwin, np.int64)
    win_hi = np.empty(nwin, np.int64)
    for wi in range(nwin):
        bk, w = divmod(wi, wpb)
        lo = bk * P + w * WIN
        hi = min(lo + WIN, npc)
        win_lo[wi] = min(lo, npc)
        win_hi[wi] = max(min(hi, npc), win_lo[wi])

    # per (core, window, half) counts
    counts = np.zeros((n_cores, nwin, 2), np.int64)
    epos = [[None] * nwin for _ in range(n_cores)]  # edge positions per group
    for i in range(n_cores):
        base = i * npc
        lo_pos = np.searchsorted(ss, base + win_lo)
        hi_pos = np.searchsorted(ss, base + win_hi)
        for wi in range(nwin):
            e = np.arange(lo_pos[wi], hi_pos[wi])
            isB = dsrt[e] >= half
            epos[i][wi] = (e[~isB], e[isB])
            counts[i, wi, 0] = len(e) - isB.sum()
            counts[i, wi, 1] = isB.sum()

    ntw = np.ceil(counts / P).astype(np.int64).max(axis=0)  # [nwin, 2]
    ntw[:, 0] = np.maximum(ntw[:, 0], 1)  # ensure >=1 tile per window
    T = int(ntw.sum())

    # per-block tile order: all A tiles (w ascending), then all B tiles
    # global tile id t -> (bk, half, w, j); also per-half slot order
    # (block-major, matching gather instruction layout)
    tile_meta = []          # (bk, w, h, slot_in_half)
    blk_tiles = [[] for _ in range(nblk)]
    half_slots = [0, 0]
    for bk in range(nblk):
        for h in (0, 1):
            for w in range(wpb):
                wi = bk * wpb + w
                for j in range(int(ntw[wi, h])):
                    t = len(tile_meta)
                    tile_meta.append((bk, w, h, half_slots[h]))
                    blk_tiles[bk].append(t)
                    half_slots[h] += 1
    SA, SB = half_slots
    pk.SA, pk.SB = SA, SB

    srcrel = np.full((n_cores, P, T), 100.0, np.float32)
    worig = np.full((n_cores, P, T), -1, np.int64)
    wsrcg = np.full((n_cores, P, T), 0, np.int64)
    idxh = [np.zeros((n_cores, sh * P), np.int64) for sh in (SA, SB)]

    for i in range(n_cores):
        for wi in range(nwin):
            bk, w = divmod(wi, wpb)
            base_node = i * npc + win_lo[wi]
            for h in (0, 1):
                e = epos[i][wi][h]
                c = len(e)
                if ntw[wi, h] == 0:
                    continue
                # tiles of (bk,w,h): consecutive tile ids & half slots
                ts = [t for t in blk_tiles[bk]
                      if tile_meta[t][1] == w and tile_meta[t][2] == h]
                s0 = tile_meta[ts[0]][3]
                k = np.arange(c)
                tloc = np.array([ts[j] for j in range(len(ts))])
                srcrel[i, k % P, tloc[k // P]] = \
                    (ss[e] - base_node).astype(np.float32)
                worig[i, k % P, tloc[k // P]] = perm[e]
                wsrcg[i, k % P, tloc[k // P]] = ss[e]
                idxh[h][i, s0 * P + k] = dsrt[e] - h * half

    # wrap indices: idxw[p, s] (p in 0..16) = idx[s*16+p], replicated x8
    def wrap(flat):
        n = flat.shape[1]
        w16 = flat.reshape(flat.shape[0], n // 16, 16)
        out = np.zeros((flat.shape[0], P, n // 16), np.int16)
        for g in range(8):
            out[:, g * 16:(g + 1) * 16, :] = np.transpose(
                w16, (0, 2, 1)).astype(np.int16)
        return out

    pk.idxwA = wrap(idxh[0])
    pk.idxwB = wrap(idxh[1])

    rowidx1 = np.zeros((n_cores, P, nblk), np.int32)
    rowidx2 = np.zeros((n_cores, P, nblk), np.int32)
    for i in range(n_cores):
        for bk in range(nblk):
            rows = np.minimum(bk * P + np.arange(P), npc - 1)
            rowidx1[i, :, bk] = (i * npc + rows).astype(np.int32)
            rowidx2[i, :, bk] = rows.astype(np.int32)

    pk.N, pk.E, pk.npc, pk.nblk, pk.nwin, pk.wpb, pk.T = \
        N, E, npc, nblk, nwin, wpb, T
    pk.n_cores = n_cores
    pk.nfeat = nfeat
    pk.half = half
    pk.ntw = ntw
    pk.tile_meta = tile_meta
    pk.blk_tiles = blk_tiles
    pk.srcrel = srcrel
    pk.worig, pk.wsrcg = worig, wsrcg
    pk.rowidx1, pk.rowidx2 = rowidx1, rowidx2
    pk.gblk = math.ceil(N / P)
    # gather groups: blocks [g*GRP, ...): per (group, half) slot ranges
    pk.ngrp = math.ceil(nblk / GRP)
    pk.grp_rng = []  # (blk0, blk1, a0, a1, b0, b1)
    for g in range(pk.ngrp):
        b0, b1 = g * GRP, min((g + 1) * GRP, nblk)
        ts = [t for bk in range(b0, b1) for t in blk_tiles[bk]]
        aslots = [tile_meta[t][3] for t in ts if tile_meta[t][2] == 0]
        bslots = [tile_meta[t][3] for t in ts if tile_meta[t][2] == 1]
        a0 = min(aslots) if aslots else 0
        a1 = max(aslots) + 1 if aslots else 0
        bb0 = min(bslots) if bslots else 0
        bb1 = max(bslots) + 1 if bslots else 0
        pk.grp_rng.append((b0, b1, a0, a1, bb0, bb1))
    pk.max_ga = max(r[3] - r[2] for r in pk.grp_rng)
    pk.max_gb = max(r[5] - r[4] for r in pk.grp_rng)
    return pk


# ----------------------------------------------------------------------------
# bass program
# ----------------------------------------------------------------------------

def build_program(pk, nclass):
    F = pk.nfeat
    assert F == 128
    N, T, nblk, npc = pk.N, pk.T, pk.nblk, pk.npc
    NCLS = nclass
    half = pk.half

    nc = bacc.Bacc("TRN2", target_bir_lowering=False, debug=False,
                   num_devices=pk.n_cores)

    # ---- I/O ----
    xT_d = nc.dram_tensor("xT", [F, N], F32, kind="ExternalInput")
    wext1_d = nc.dram_tensor("wext1", [F, 130], F32, kind="ExternalInput")
    wext2_d = nc.dram_tensor("wext2", [F, 130], F32, kind="ExternalInput")
    fcw_d = nc.dram_tensor("fcw", [F, NCLS], F32, kind="ExternalInput")
    fcb_d = nc.dram_tensor("fcb", [1, NCLS], F32, kind="ExternalInput")
    ident_d = nc.dram_tensor("ident", [P, P], F32, kind="ExternalInput")
    iota_d = nc.dram_tensor("iota32", [P, WIN], F32, kind="ExternalInput")
    ab1_d = nc.dram_tensor("ab1", [P, 1], F32, kind="ExternalInput")
    ab2_d = nc.dram_tensor("ab2", [P, 1], F32, kind="ExternalInput")
    idxa_d = nc.dram_tensor("idxwA", [P, pk.SA * P // 16], I16,
                            kind="ExternalInput")
    idxb_d = nc.dram_tensor("idxwB", [P, max(pk.SB, 1) * P // 16], I16,
                            kind="ExternalInput")
    srel_d = nc.dram_tensor("srcrel", [P, T], F32, kind="ExternalInput")
    ri1_d = nc.dram_tensor("rowidx1", [P, nblk], I32, kind="ExternalInput")
    ri2_d = nc.dram_tensor("rowidx2", [P, nblk], I32, kind="ExternalInput")

    out_logits = nc.dram_tensor("out_logits", [P, nblk * NCLS], F32,
                                kind="ExternalOutput")
    out_w = nc.dram_tensor("out_w", [P, T], F32, kind="ExternalOutput")
    out_den = nc.dram_tensor("out_den", [P, nblk], F32, kind="ExternalOutput")

    with tile.TileContext(nc) as tc:
        with (
            tc.tile_pool(name="const", bufs=1) as cpool,
            tc.tile_pool(name="gat", bufs=2) as gpool,
            tc.tile_pool(name="selp", bufs=3) as spool,
            tc.tile_pool(name="stage", bufs=3) as stpool,
            tc.tile_pool(name="big", bufs=1) as bigpool,
            tc.tile_pool(name="aug", bufs=3) as augpool,
            tc.tile_pool(name="xc", bufs=3) as xcpool,
            tc.tile_pool(name="psA", bufs=3, space="PSUM") as psA,
            tc.tile_pool(name="psB", bufs=3, space="PSUM") as psB,
            tc.tile_pool(name="dram", bufs=1, space="DRAM") as dpool,
        ):
            table1 = dpool.tile([N, TWU], U16, name="table1",
                                allow_tmpbuf=True)
            shard2 = dpool.tile([npc, TWU], U16, name="shard2",
                                allow_tmpbuf=True)
            # NOTE: keep table2 Local — the dma_gather ucode reads it, and
            # Shared-scratchpad address translation crashes that path.
            table2 = dpool.tile([N, TWU], U16, name="table2",
                                allow_tmpbuf=True)

            wext1 = cpool.tile_from(wext1_d[:, :])
            wext2 = cpool.tile_from(wext2_d[:, :])
            fcw = cpool.tile_from(fcw_d[:, :])
            fcb = cpool.tile_from(fcb_d[:, :])
            ident = cpool.tile_from(ident_d[:, :])
            iota32 = cpool.tile_from(iota_d[:, :])
            ab1 = cpool.tile_from(ab1_d[:, :])
            ab2 = cpool.tile_from(ab2_d[:, :])
            idxa = cpool.tile_from(idxa_d[:, :])
            idxb = cpool.tile_from(idxb_d[:, :])
            srel = cpool.tile_from(srel_d[:, :])
            ri1 = cpool.tile_from(ri1_d[:, :])
            ri2 = cpool.tile_from(ri2_d[:, :])
            ones_row = cpool.tile([1, P], F32)
            nc.vector.memset(ones_row[:, :], 1.0)

            # dma_gather lives in the mlp gpsimd ucode library
            nc.gpsimd.load_library(library_config.mlp)

            es_sb = bigpool.tile([P, nblk], F32)
            w_stage = bigpool.tile([P, T], F32)
            denw = bigpool.tile([P, nblk], F32)
            lg_stage = bigpool.tile([P, nblk * NCLS], F32)
            lg_tmp = bigpool.tile([P, nblk * NCLS], F32)

            def build_aug(ps, engine_act):
                """Assemble a 512B-row aug tile from transform PSUM
                [h(0:128) f32, s_src(128), s_dst(129)]."""
                aug = augpool.tile([P, TWU], U16, tag="aug")
                aug_bf = aug.bitcast(BF16)
                if engine_act:
                    nc.scalar.copy(out=aug_bf[:, 0:128], in_=ps[:, 0:128])
                else:
                    nc.vector.tensor_copy(out=aug_bf[:, 0:128],
                                          in_=ps[:, 0:128])
                nc.vector.memset(aug_bf[:, 128:130], 1.0)
                nc.vector.tensor_copy(
                    out=aug[:, COL_SSRC_U:COL_SSRC_U + 2].bitcast(F32),
                    in_=ps[:, 128:129])
                nc.vector.tensor_copy(
                    out=aug[:, COL_SDST_U:COL_SDST_U + 2].bitcast(F32),
                    in_=ps[:, 129:130])
                nc.vector.memset(aug[:, 134:256], 0)
                return aug

            # ================= layer-1 table (replicated) =================
            XCH = 8
            gblk = pk.gblk
            for g0 in range(0, gblk, XCH):
                g1 = min(g0 + XCH, gblk)
                wd = min(g1 * P, N) - g0 * P
                xch = xcpool.tile([P, XCH * P], F32, tag="xch")
                nc.sync.dma_start(out=xch[:, :wd],
                                  in_=xT_d[:, g0 * P:g0 * P + wd])
                for g in range(g0, g1):
                    nbg = min(P, N - g * P)
                    hps = psB.tile([P, 130], F32, tag="psB")
                    nc.tensor.matmul(
                        out=hps[:nbg, 0:130],
                        lhsT=xch[:, (g - g0) * P:(g - g0) * P + nbg],
                        rhs=wext1[:, :], start=True, stop=True)
                    aug = build_aug(hps, g % 2 == 0)
                    nc.sync.dma_start(out=table1[g * P:g * P + nbg, :],
                                      in_=aug[:nbg, :])

            # ================= edge phases =================
            def edge_layer(layer, table_ap, rowidx, es_rows_src, ab):
                es_raw = stpool.tile([P, 2 * nblk], U16, tag="esraw", bufs=1)
                for bk in range(nblk):
                    nc.gpsimd.indirect_dma_start(
                        out=es_raw[:, 2 * bk:2 * bk + 2], out_offset=None,
                        in_=es_rows_src[:, :],
                        in_offset=bass.IndirectOffsetOnAxis(
                            ap=rowidx[:, bk:bk + 1], axis=0),
                        element_offset=COL_SSRC_U)
                nc.scalar.activation(out=es_sb[:, :],
                                     in_=es_raw[:, :].bitcast(F32),
                                     func=mybir.ActivationFunctionType.Exp)

                for g, (b0, b1, a0, a1, bb0, bb1) in enumerate(pk.grp_rng):
                    na, nb_ = a1 - a0, bb1 - bb0
                    hga = gpool.tile([P, pk.max_ga * TWU], U16, tag="hga")
                    hgb = gpool.tile([P, max(pk.max_gb, 1) * TWU], U16,
                                     tag="hgb")
                    if na > 0:
                        nc.gpsimd.dma_gather(
                            out_ap=hga[:, :na * TWU].rearrange(
                                "p (t d) -> p t d", d=TWU),
                            in_ap=table_ap[0:half, :],
                            idxs_ap=idxa[:, a0 * 8:a1 * 8],
                            num_idxs=na * P, num_idxs_reg=na * P,
                            elem_size=TWU, single_packet=False)
                    if nb_ > 0:
                        nc.gpsimd.dma_gather(
                            out_ap=hgb[:, :nb_ * TWU].rearrange(
                                "p (t d) -> p t d", d=TWU),
                            in_ap=table_ap[half:N, :],
                            idxs_ap=idxb[:, bb0 * 8:bb1 * 8],
                            num_idxs=nb_ * P, num_idxs_reg=nb_ * P,
                            elem_size=TWU, single_packet=False)
                    for bk in range(b0, b1):
                        edge_block(layer, bk, hga, hgb, a0, bb0, ab)

            def edge_block(layer, bk, hga, hgb, ga0, gb0, ab):
                tiles = pk.blk_tiles[bk]
                t0 = tiles[0]
                TB = len(tiles)
                metas = [pk.tile_meta[t] for t in tiles]
                nA = sum(1 for m in metas if m[2] == 0)

                def hg_view(m):
                    hg, base = (hga, ga0) if m[2] == 0 else (hgb, gb0)
                    s = m[3] - base
                    return hg, s

                # es broadcast
                esb = psA.tile([P, P], F32, tag="psA")
                nc.tensor.transpose(
                    out=esb[:, :],
                    in_=es_sb[:, bk:bk + 1].to_broadcast([P, P]),
                    identity=ident[:, :])

                # sel = (srcrel == iota)
                sel = spool.tile([P, TB * WIN], F32, tag="sel")
                sel3 = sel.rearrange("p (t j) -> p t j", j=WIN)
                srel3 = srel[:, t0:t0 + TB].unsqueeze(2) \
                    .to_broadcast([P, TB, WIN])
                iota3 = iota32[:, :].unsqueeze(1).to_broadcast([P, TB, WIN])
                nc.vector.tensor_tensor(out=sel3, in0=srel3, in1=iota3,
                                        op=mybir.AluOpType.is_equal)

                # prod = sel * es_bcast, per (half, window) run
                prod = spool.tile([P, TB * WIN], F32, tag="prod")
                prod3 = prod.rearrange("p (t j) -> p t j", j=WIN)
                r = 0
                while r < TB:
                    w = metas[r][1]
                    r2 = r
                    while r2 < TB and metas[r2][1] == w and \
                            metas[r2][2] == metas[r][2]:
                        r2 += 1
                    esw = esb[:, w * WIN:(w + 1) * WIN].unsqueeze(1) \
                        .to_broadcast([P, r2 - r, WIN])
                    nc.vector.tensor_tensor(
                        out=prod3[:, r:r2, :], in0=sel3[:, r:r2, :],
                        in1=esw, op=mybir.AluOpType.mult)
                    r = r2

                # per-edge exp(ssrc)
                essrc = stpool.tile([P, TB], F32, tag="essrc")
                nc.vector.tensor_reduce(out=essrc[:, :], in_=prod3,
                                        axis=mybir.AxisListType.X,
                                        op=mybir.AluOpType.add)
                nc.vector.tensor_scalar_max(out=essrc[:, :], in0=essrc[:, :],
                                            scalar1=1e-30)
                ssrc = stpool.tile([P, TB], F32, tag="ssrc")
                nc.scalar.activation(out=ssrc[:, :], in_=essrc[:, :],
                                     func=mybir.ActivationFunctionType.Ln)

                # e = ssrc + bias + sdst (A part then B part)
                epre = stpool.tile([P, TB], F32, tag="epre")
                for h, lo, hi in ((0, 0, nA), (1, nA, TB)):
                    if hi == lo:
                        continue
                    hg, s = hg_view(metas[lo])
                    hg3u = hg.rearrange("p (t d) -> p t d", d=TWU)
                    sdst = hg3u[:, s:s + hi - lo,
                                COL_SDST_U:COL_SDST_U + 2].bitcast(F32) \
                        .squeeze(2)
                    nc.vector.scalar_tensor_tensor(
                        out=epre[:, lo:hi], in0=ssrc[:, lo:hi],
                        scalar=ab[:, :], in1=sdst,
                        op0=mybir.AluOpType.add, op1=mybir.AluOpType.add)
                lr = stpool.tile([P, TB], F32, tag="lr")
                nc.vector.scalar_tensor_tensor(
                    out=lr[:, :], in0=epre[:, :], scalar=LEAKY,
                    in1=epre[:, :], op0=mybir.AluOpType.mult,
                    op1=mybir.AluOpType.max)
                if layer == 2:
                    nc.scalar.activation(out=w_stage[:, t0:t0 + TB],
                                         in_=lr[:, :],
                                         func=mybir.ActivationFunctionType.Exp)
                t2 = stpool.tile([P, TB], F32, tag="t2")
                nc.vector.tensor_tensor(out=t2[:, :], in0=lr[:, :],
                                        in1=ssrc[:, :],
                                        op=mybir.AluOpType.subtract)
                ratio = stpool.tile([P, TB], F32, tag="ratio")
                nc.scalar.activation(out=ratio[:, :], in_=t2[:, :],
                                     func=mybir.ActivationFunctionType.Exp)

                # W (bf16) and aggregation matmuls
                Wm = spool.tile([P, TB * WIN], BF16, tag="Wm")
                agg = psB.tile([P, 130], F32, tag="psB")
                for k in range(TB):
                    nc.scalar.activation(
                        out=Wm[:, k * WIN:(k + 1) * WIN],
                        in_=prod[:, k * WIN:(k + 1) * WIN],
                        func=mybir.ActivationFunctionType.Copy,
                        scale=ratio[:, k:k + 1])
                # emit matmuls window-major so PSUM accumulation groups for
                # the per-window 32-row slices open and close sequentially
                for w in range(pk.wpb):
                    ks = [k for k, m in enumerate(metas) if m[1] == w]
                    for kk, k in enumerate(ks):
                        hg, s = hg_view(metas[k])
                        hg_bf3 = hg.bitcast(BF16).rearrange(
                            "p (t d) -> p t d", d=TWU)
                        nc.tensor.matmul(
                            out=agg[w * WIN:(w + 1) * WIN, 0:RHSW],
                            lhsT=Wm[:, k * WIN:(k + 1) * WIN],
                            rhs=hg_bf3[:, s, 0:RHSW],
                            start=(kk == 0), stop=(kk == len(ks) - 1),
                            tile_position=(0, w * WIN))

                # epilogue
                nb = min(P, npc - bk * P)
                dsafe = stpool.tile([P, 1], F32, tag="dsafe")
                nc.vector.tensor_scalar_max(
                    out=dsafe[:, :], in0=agg[:, COL_ONE_BF:COL_ONE_BF + 1],
                    scalar1=1e-30)
                recip = stpool.tile([P, 1], F32, tag="recip")
                nc.vector.reciprocal(out=recip[:, :], in_=dsafe[:, :])
                h2f = stpool.tile([P, P], F32, tag="h2f")
                nc.scalar.activation(out=h2f[:, :], in_=agg[:, 0:P],
                                     func=mybir.ActivationFunctionType.Relu,
                                     scale=recip[:, :])
                tp = psA.tile([P, P], F32, tag="psA")
                nc.tensor.transpose(out=tp[:, :], in_=h2f[:, :],
                                    identity=ident[:, :])
                h2t = stpool.tile([P, P], F32, tag="h2t")
                nc.vector.tensor_copy(out=h2t[:, :], in_=tp[:, :])

                if layer == 1:
                    bps = psB.tile([P, 130], F32, tag="psB")
                    nc.tensor.matmul(out=bps[:, 0:130], lhsT=h2t[:, :],
                                     rhs=wext2[:, :], start=True, stop=True)
                    aug = build_aug(bps, True)
                    nc.sync.dma_start(out=shard2[bk * P:bk * P + nb, :],
                                      in_=aug[:nb, :])
                else:
                    nc.vector.tensor_copy(out=denw[:, bk:bk + 1],
                                          in_=dsafe[:, :])
                    lps = psA.tile([P, P], F32, tag="psA")
                    nc.tensor.matmul(out=lps[:, 0:NCLS], lhsT=h2t[:, :],
                                     rhs=fcw[:, :], start=True, stop=False)
                    nc.tensor.matmul(out=lps[:, 0:NCLS], lhsT=ones_row[:, :],
                                     rhs=fcb[:, :], start=False, stop=True)
                    nc.vector.tensor_copy(
                        out=lg_stage[:, bk * NCLS:(bk + 1) * NCLS],
                        in_=lps[:, 0:NCLS])

            # ---- layer 1 ----
            edge_layer(1, table1, ri1, table1, ab1)

            # ---- allgather shard2 -> table2 ----
            if pk.n_cores > 1:
                nc.gpsimd.collective_compute(
                    "AllGather", mybir.AluOpType.bypass,
                    replica_groups=[list(range(pk.n_cores))],
                    ins=[shard2[:, :]],
                    outs=[table2[0:N, :]],
                )
            else:
                nc.sync.dma_start(out=table2[0:N, :], in_=shard2[:, :])

            # ---- layer 2 ----
            edge_layer(2, table2, ri2, shard2, ab2)

            # ---- log softmax over classes ----
            lg3 = lg_stage.rearrange("p (b c) -> p b c", c=NCLS)
            tmp3 = lg_tmp.rearrange("p (b c) -> p b c", c=NCLS)
            rmax = stpool.tile([P, nblk], F32, tag="rmax", bufs=1)
            nc.vector.tensor_reduce(out=rmax[:, :], in_=lg3,
                                    axis=mybir.AxisListType.X,
                                    op=mybir.AluOpType.max)
            rmax3 = rmax[:, :].unsqueeze(2).to_broadcast([P, nblk, NCLS])
            nc.vector.tensor_tensor(out=tmp3, in0=lg3, in1=rmax3,
                                    op=mybir.AluOpType.subtract)
            ez = stpool.tile([P, nblk * NCLS], F32, tag="ez", bufs=1)
            nc.scalar.activation(out=ez[:, :], in_=lg_tmp[:, :],
                                 func=mybir.ActivationFunctionType.Exp)
            esum = stpool.tile([P, nblk], F32, tag="esum", bufs=1)
            nc.vector.tensor_reduce(
                out=esum[:, :],
                in_=ez.rearrange("p (b c) -> p b c", c=NCLS),
                axis=mybir.AxisListType.X, op=mybir.AluOpType.add)
            lse = stpool.tile([P, nblk], F32, tag="lse", bufs=1)
            nc.scalar.activation(out=lse[:, :], in_=esum[:, :],
                                 func=mybir.ActivationFunctionType.Ln)
            lse3 = lse[:, :].unsqueeze(2).to_broadcast([P, nblk, NCLS])
            nc.vector.tensor_tensor(out=lg3, in0=tmp3, in1=lse3,
                                    op=mybir.AluOpType.subtract)

            nc.sync.dma_start(out=out_logits[:, :], in_=lg_stage[:, :])
            nc.sync.dma_start(out=out_w[:, :], in_=w_stage[:, :])
            nc.sync.dma_start(out=out_den[:, :], in_=denw[:, :])

    nc.compile()
    return nc


# ----------------------------------------------------------------------------
# host driver
# ----------------------------------------------------------------------------

def make_inputs(pk, x, W1, a1_w, a1_b, W2, a2_w, a2_b, fcW, fcb):
    F = pk.nfeat

    def wext(W, a_w):
        return np.concatenate(
            [W, W @ a_w[:F, :1], W @ a_w[F:, :1]], axis=1
        ).astype(np.float32)

    common = {
        "xT": np.ascontiguousarray(np.asarray(x).T.astype(np.float32)),
        "wext1": wext(np.asarray(W1), np.asarray(a1_w)),
        "wext2": wext(np.asarray(W2), np.asarray(a2_w)),
        "fcw": np.asarray(fcW, np.float32).reshape(F, -1),
        "fcb": np.asarray(fcb, np.float32).reshape(1, -1),
        "ident": np.eye(P, dtype=np.float32),
        "iota32": np.tile(np.arange(WIN, dtype=np.float32), (P, 1)),
        "ab1": np.full((P, 1), np.float32(np.asarray(a1_b).reshape(-1)[0])),
        "ab2": np.full((P, 1), np.float32(np.asarray(a2_b).reshape(-1)[0])),
        "rowidx2": pk.rowidx2[0],
    }
    in_maps = []
    for i in range(pk.n_cores):
        m = dict(common)
        m["idxwA"] = np.ascontiguousarray(pk.idxwA[i])
        m["idxwB"] = np.ascontiguousarray(
            pk.idxwB[i] if pk.SB > 0
            else np.zeros((P, P // 16), np.int16))
        m["srcrel"] = np.ascontiguousarray(pk.srcrel[i])
        m["rowidx1"] = np.ascontiguousarray(pk.rowidx1[i])
        in_maps.append(m)
    return in_maps


def assemble_outputs(pk, results, nclass):
    N, E, npc, nblk, T = pk.N, pk.E, pk.npc, pk.nblk, pk.T
    logits = np.empty((N, nclass), np.float32)
    w_all = []
    for i in range(pk.n_cores):
        r = results[i]
        lg = r["out_logits"].reshape(P, nblk, nclass).transpose(1, 0, 2) \
            .reshape(nblk * P, nclass)
        logits[i * npc:(i + 1) * npc] = lg[:npc]
        w_all.append(r["out_w"].reshape(P, T))
    # exact denominators on host from per-edge w
    den_full = np.zeros(N, np.float32)
    alpha = np.zeros(E, np.float32)
    for i in range(pk.n_cores):
        mask = pk.worig[i] >= 0
        np.add.at(den_full, pk.wsrcg[i][mask], w_all[i][mask])
    for i in range(pk.n_cores):
        mask = pk.worig[i] >= 0
        alpha[pk.worig[i][mask]] = (
            w_all[i][mask] / den_full[pk.wsrcg[i][mask]]).astype(np.float32)
    return logits, alpha


_CACHE = {}


def run_gat(x, src, dst, W1, a1_w, a1_b, W2, a2_w, a2_b, fcW, fcb,
            n_cores=8, trace=False):
    N, F = x.shape
    E = src.shape[0]
    nclass = np.asarray(fcW).shape[1]
    key = (N, E, F, nclass, n_cores,
           int(np.asarray(src[:64]).sum()), int(np.asarray(dst[:64]).sum()))
    if key in _CACHE:
        pk, nc = _CACHE[key]
    else:
        pk = pack_edges(np.asarray(src), np.asarray(dst), N, n_cores, F)
        nc = build_program(pk, nclass)
        _CACHE[key] = (pk, nc)
    in_maps = make_inputs(pk, x, W1, a1_w, a1_b, W2, a2_w, a2_b, fcW, fcb)
    res = bass_utils.run_bass_kernel_spmd(
        nc, in_maps, core_ids=list(range(n_cores)), trace=trace)
    logits, alpha = assemble_outputs(pk, res.results, nclass)
    return logits, alpha, res


def kernel(x, src, dst, W1, a1_w, a1_b, W2, a2_w, a2_b, fcW, fcb):
    logits, alpha, _ = run_gat(
        np.asarray(x, np.float32), np.asarray(src), np.asarray(dst),
        np.asarray(W1), np.asarray(a1_w), np.asarray(a1_b),
        np.asarray(W2), np.asarray(a2_w), np.asarray(a2_b),
        np.asarray(fcW), np.asarray(fcb), n_cores=8, trace=False)
    return logits, alpha
